# revision 11
# baseline (speedup 1.0000x reference)
"""Efficient Channel Attention kernel for 8 Trainium2 NeuronCores.

Problem (B=4, N=4096, C=1024, H=4, HD=256):
    qkv = x @ Wqkv.T                 -> q,k,v per head, [HD, N] layout
    q,k l2-normalized over N; scores = (q*temp) @ k.T   [HD, HD] per (b,h)
    attn = softmax(scores, -1); out = attn @ v; y = out @ Wproj.T + bproj + x

Sharding: core = (batch b, token-half); only cross-core data is the
token-contracted Grams k^T q + q/k sumsq, AllReduce'd (f32, ~1MB) within
the core pair sharing a batch.

All heavy GEMMs run as fp8e4m3 DoubleRow matmuls (2 slab-contraction per
pass = 157 TF/s, 2x the f32r rate). Host pre-quantizes x to fp8 and the
weights to fp8*64 (keeps W entries out of fp8 subnormals); scales unwind in
the psum->SBUF copies. Local tokens are host-permuted t -> (t%4)*512 + t//4
so the torch transpose+reshape channel scramble makes the proj-phase moving
operands contiguous (the f32r baseline paid 1.65x for stride-4 reads).
Residual + y I/O in bf16.

PSUM tags: q0,q1,k0,k1 (4x1 bank) + gA,gB (2x2 banks) cover all phases.
"""

import numpy as np

B, N, C, H = 4, 4096, 1024, 4
HD = C // H          # 256
NCORES = 8
NL = N // 2          # 2048 tokens per core
S = C // 128         # 8 channel slabs
NT = NL // 128       # 16 token tiles
EPS = 1e-12
WS = 64.0            # host weight scale (fp8 subnormal dodge)
CCN = 128 * 2048 + 2 * C

_CACHE = {}


def _build():
    import concourse.mybir as mybir
    import concourse.tile as tile
    from concourse import bacc
    from concourse.masks import make_identity

    f32 = mybir.dt.float32
    bf16 = mybir.dt.bfloat16
    f8 = mybir.dt.float8e4
    AX = mybir.AxisListType.X
    ADD = mybir.AluOpType.add
    MULT = mybir.AluOpType.mult
    DR = mybir.MatmulPerfMode.DoubleRow
    Exp = mybir.ActivationFunctionType.Exp
    Ident = mybir.ActivationFunctionType.Identity
    Sqrt = mybir.ActivationFunctionType.Sqrt

    nc = bacc.Bacc("TRN2", target_bir_lowering=False, debug=False,
                   num_devices=NCORES)

    x8_d = nc.dram_tensor("x8", [128, S, NL], f8, kind="ExternalInput").ap()
    wqk_d = nc.dram_tensor("wqk8", [128, S, 2 * C], f8, kind="ExternalInput").ap()
    wv_d = nc.dram_tensor("wv8", [128, S, C], f8, kind="ExternalInput").ap()
    wp_d = nc.dram_tensor("wp8", [128, S, C], f8, kind="ExternalInput").ap()
    xr_d = nc.dram_tensor("xr", [C, NL], bf16, kind="ExternalInput").ap()
    tmpv_d = nc.dram_tensor("tmpv", [128, S], f32, kind="ExternalInput").ap()
    yT_d = nc.dram_tensor("yT", [C, NL], bf16, kind="ExternalOutput").ap()

    with tile.TileContext(nc) as tc:
        with (
            tc.tile_pool(name="const", bufs=1) as constp,
            tc.tile_pool(name="big", bufs=1) as bigp,
            tc.tile_pool(name="wrk", bufs=1) as wrk,
            tc.tile_pool(name="ps", bufs=1, space="PSUM") as ps,
            tc.tile_pool(name="ps2", bufs=1, space="PSUM") as ps2,
            tc.tile_pool(name="dram", bufs=1, space="DRAM") as dramp,
        ):
            PT = ["q0", "q1", "k0", "k1"]

            # ---------------- constants ----------------
            ident = constp.tile([128, 128], f32, name="ident")
            make_identity(nc, ident[:])
            tmpv_sb = constp.tile([128, S], f32, name="tmpv_sb")
            nc.sync.dma_start(tmpv_sb[:], tmpv_d[:])
            ones_sb = constp.tile([128, 1], bf16, name="ones_sb")
            nc.vector.memset(ones_sb[:], 1.0)

            # resident inputs; first slabs of x/wqk land first so A1 starts
            x8 = bigp.tile([128, S, NL], f8, name="x8")
            wqk = bigp.tile([128, S, 2 * C], f8, name="wqk")
            for s in range(S):
                nc.sync.dma_start(x8[:, s, :], x8_d[:, s, :])
                nc.sync.dma_start(wqk[:, s, :], wqk_d[:, s, :])
            wv = bigp.tile([128, S, C], f8, name="wv")
            wp = bigp.tile([128, S, C], f8, name="wp")
            xr = bigp.tile([128, S, NL], bf16, name="xr")
            # issue on the Activation DGE queue so they never block the
            # sync queue's collective payload DMAs
            for s in range(S):
                nc.scalar.dma_start(wv[:, s, :], wv_d[:, s, :])
            for s in range(S):
                nc.scalar.dma_start(wp[:, s, :], wp_d[:, s, :])
            for s in range(S):
                nc.scalar.dma_start(xr[:, s, :], xr_d[s * 128:(s + 1) * 128, :])

            # Gram accumulators: stA = heads 0,1 / stB = heads 2,3
            stA = ps2.tile([128, 1024], f32, tag="gA", name="stA")
            stB = ps2.tile([128, 1024], f32, tag="gB", name="stB")

            def st_slice(h, m):
                t = stA if h < 2 else stB
                off = (h % 2) * 512 + m * 256
                return t[:, off:off + 256]

            accq = wrk.tile([128, C], bf16, tag="accq", name="accq")
            acck = wrk.tile([128, C], bf16, tag="acck", name="acck")

            def _emit_gram(tp, qcol, kcol):
                for h in range(H):
                    for m in range(2):
                        nc.tensor.matmul(
                            st_slice(h, m),
                            kcol[:, :, h * 256 + m * 128: h * 256 + (m + 1) * 128],
                            qcol[:, :, h * 256:(h + 1) * 256],
                            start=(tp == 0), stop=(tp == NT // 2 - 1),
                            perf_mode=DR, skip_group_check=True)

            # ---------------- phase A1: q,k + Grams + sumsq ----------------
            _gram_pend = None
            for tp in range(NT // 2):
                qcol = wrk.tile([128, 2, C], f8, tag="qcol", bufs=2,
                                name=f"qcol{tp}")
                kcol = wrk.tile([128, 2, C], f8, tag="kcol", bufs=2,
                                name=f"kcol{tp}")
                for i2 in range(2):
                    tidx = tp * 2 + i2
                    t0 = tidx * 128
                    qp0 = ps.tile([128, 512], f32, tag="q0", name="qp0")
                    qp1 = ps.tile([128, 512], f32, tag="q1", name="qp1")
                    kp0 = ps.tile([128, 512], f32, tag="k0", name="kp0")
                    kp1 = ps.tile([128, 512], f32, tag="k1", name="kp1")
                    for kt2 in range(4):
                        lhs = x8[:, 2 * kt2:2 * kt2 + 2, t0:t0 + 128]
                        w2 = wqk[:, 2 * kt2:2 * kt2 + 2, :]
                        fl, ll = (kt2 == 0), (kt2 == 3)
                        nc.tensor.matmul(qp0[:], lhs, w2[:, :, 0:512],
                                         start=fl, stop=ll, perf_mode=DR)
                        nc.tensor.matmul(qp1[:], lhs, w2[:, :, 512:1024],
                                         start=fl, stop=ll, perf_mode=DR)
                        nc.tensor.matmul(kp0[:], lhs, w2[:, :, 1024:1536],
                                         start=fl, stop=ll, perf_mode=DR)
                        nc.tensor.matmul(kp1[:], lhs, w2[:, :, 1536:2048],
                                         start=fl, stop=ll, perf_mode=DR)
                    # fp8 copies for the Gram (vector); squares of the fp8
                    # values for the norms (scalar q / gpsimd k, from SBUF —
                    # gpsimd cannot touch PSUM), accumulate sumsq
                    nc.vector.tensor_scalar_mul(qcol[:, i2, 0:512], qp0[:],
                                                1.0 / WS)
                    nc.vector.tensor_scalar_mul(qcol[:, i2, 512:1024], qp1[:],
                                                1.0 / WS)
                    nc.vector.tensor_scalar_mul(kcol[:, i2, 0:512], kp0[:],
                                                1.0 / WS)
                    nc.vector.tensor_scalar_mul(kcol[:, i2, 512:1024], kp1[:],
                                                1.0 / WS)
                    sq = wrk.tile([128, C], bf16, tag="sq", bufs=2,
                                  name=f"sq{tidx}")
                    sk = wrk.tile([128, C], bf16, tag="sk", bufs=2,
                                  name=f"sk{tidx}")
                    nc.scalar.square(sq[:], qcol[:, i2, :])
                    nc.scalar.square(sk[:], kcol[:, i2, :])
                    if tidx == 0:
                        nc.gpsimd.tensor_copy(accq[:], sq[:])
                        nc.gpsimd.tensor_copy(acck[:], sk[:])
                    else:
                        nc.gpsimd.tensor_add(accq[:], accq[:], sq[:])
                        nc.gpsimd.tensor_add(acck[:], acck[:], sk[:])
                # Gram for the PREVIOUS pair: its fp8 copies finished long
                # ago, so the PE never stalls on fresh vector output
                if tp > 0:
                    _emit_gram(tp - 1, *_gram_pend)
                _gram_pend = (qcol, kcol)
            _emit_gram(NT // 2 - 1, *_gram_pend)

            # sumsq rows: [1, 512] ones-matmuls into the freed qk psum slots
            ss_ps = []
            for i, (src, lo) in enumerate([(accq, 0), (accq, 512),
                                           (acck, 0), (acck, 512)]):
                sp = ps.tile([1, 512], f32, tag=PT[i], name=f"ss{i}")
                nc.tensor.matmul(sp[:], ones_sb[:], src[:, lo:lo + 512],
                                 start=True, stop=True)
                ss_ps.append(sp)

            # SBUF bounces for the collective (DMA cannot read PSUM)
            # NOTE: bf16 collective hangs the runtime in-context (probed OK
            # bare); f32 collective is reliable — keep f32.
            ccdt = f32
            stA_sb = wrk.tile([128, 1024], ccdt, tag="ccA", name="stA_sb")
            stB_sb = wrk.tile([128, 1024], ccdt, tag="ccB", name="stB_sb")
            nc.vector.tensor_copy(stA_sb[:], stA[:])
            nc.vector.tensor_copy(stB_sb[:], stB[:])
            ss_sb = []
            for i in range(4):
                sb = wrk.tile([1, 512], ccdt, tag=f"ssb{i}", name=f"ssb{i}")
                nc.vector.tensor_copy(sb[:], ss_ps[i][:])
                ss_sb.append(sb)

            # ---------------- AllReduce over batch-pairs (bf16) -----------
            cc_in = dramp.tile([CCN], ccdt, name="cc_in")
            cc_out = dramp.tile([CCN], ccdt, name="cc_out")
            nc.sync.dma_start(
                cc_in[0:131072].rearrange("(p f) -> p f", p=128), stA_sb[:])
            nc.sync.dma_start(
                cc_in[131072:262144].rearrange("(p f) -> p f", p=128), stB_sb[:])
            for i in range(4):
                nc.sync.dma_start(
                    cc_in[262144 + i * 512: 262144 + (i + 1) * 512]
                    .rearrange("(a f) -> a f", a=1), ss_sb[i][:])
            nc.gpsimd.collective_compute(
                "AllReduce", ADD,
                replica_groups=[[0, 1], [2, 3], [4, 5], [6, 7]],
                ins=[cc_in.opt()], outs=[cc_out.opt()])
            strA = wrk.tile([128, 1024], ccdt, tag="ccA", name="strA")
            strB = wrk.tile([128, 1024], ccdt, tag="ccB", name="strB")
            nc.sync.dma_start(
                strA[:], cc_out[0:131072].rearrange("(p f) -> p f", p=128))
            nc.sync.dma_start(
                strB[:], cc_out[131072:262144].rearrange("(p f) -> p f", p=128))
            ssred = constp.tile([128, 16], ccdt, name="ssred")
            nc.sync.dma_start(
                ssred[:],
                cc_out[262144:262144 + 2048].rearrange("(j p) -> p j", p=128))

            def str_slice(h, m):
                t = strA if h < 2 else strB
                off = (h % 2) * 512 + m * 256
                return t[:, off:off + 256]

            # ---------------- phase A2: v (overlaps the collective) -------
            v_sb = [bigp.tile([128, 2, NL], f8, name=f"v{h}") for h in range(H)]
            for vb in range(8):
                h, iv = vb // 2, vb % 2
                if vb % 2 == 0:
                    vps = [ps.tile([128, 512], f32, tag=PT[tc],
                                   name=f"vp{vb}_{tc}")[:] for tc in range(4)]
                else:
                    vA = ps2.tile([128, 1024], f32, tag="gA", name=f"vA{vb}")
                    vB = ps2.tile([128, 1024], f32, tag="gB", name=f"vB{vb}")
                    vps = [vA[:, 0:512], vA[:, 512:1024],
                           vB[:, 0:512], vB[:, 512:1024]]
                for kt2 in range(4):
                    fl, ll = (kt2 == 0), (kt2 == 3)
                    wvs = wv[:, 2 * kt2:2 * kt2 + 2, vb * 128:(vb + 1) * 128]
                    for tc in range(4):
                        nc.tensor.matmul(
                            vps[tc], wvs,
                            x8[:, 2 * kt2:2 * kt2 + 2, tc * 512:(tc + 1) * 512],
                            start=fl, stop=ll, perf_mode=DR)
                for tc in range(4):
                    nc.vector.tensor_scalar_mul(
                        v_sb[h][:, iv, tc * 512:(tc + 1) * 512], vps[tc],
                        1.0 / WS)

            # ---------------- phase B: normalize + softmax + attn@v -------
            # rq = temp/max(||q||,eps), rk = 1/max(||k||,eps) per channel:
            # rqk [128, 16]: cols 0-7 = rq (chan j*128+p), 8-15 = rk.
            rqk = constp.tile([128, 16], f32, name="rqk")
            nc.scalar.activation(rqk[:], ssred[:], Sqrt)
            nc.vector.tensor_scalar_max(rqk[:], rqk[:], EPS)
            nc.vector.reciprocal(rqk[:], rqk[:])
            nc.vector.tensor_mul(rqk[:, 0:8], rqk[:, 0:8], tmpv_sb[:])
            # preload the Exp activation table while A2 still owns the PE
            scrap = constp.tile([1, 1], f32, name="scrap")
            nc.scalar.activation(scrap[:], rqk[0:1, 0:1], Exp)

            outT = [bigp.tile([128, 2, NL], f8, name=f"ot{h}") for h in range(H)]
            # pass 1: per-head softmax -> fp8 attn^T (all heads before pass 2
            # so the scalar engine's activation table switches only once)
            atns, recips = [], []
            for h in range(H):
                # Gram^T rows d scaled by rk[d]
                sth = wrk.tile([128, 512], f32, tag="sth", bufs=2,
                               name=f"sth{h}")
                for m in range(2):
                    nc.gpsimd.tensor_scalar_mul(
                        sth[:, m * 256:(m + 1) * 256], str_slice(h, m),
                        rqk[:, 8 + 2 * h + m: 9 + 2 * h + m])
                spm = ps.tile([128, 512], f32, tag="q0", name=f"spm{h}")
                for mc in range(2):
                    for md in range(2):
                        nc.tensor.transpose(
                            spm[:, mc * 256 + md * 128: mc * 256 + (md + 1) * 128],
                            sth[:, md * 256 + mc * 128: md * 256 + (mc + 1) * 128],
                            ident[:])
                sft = wrk.tile([128, 512], f32, tag="sft", bufs=2,
                               name=f"sft{h}")
                for mc in range(2):
                    nc.vector.tensor_scalar_mul(
                        sft[:, mc * 256:(mc + 1) * 256],
                        spm[:, mc * 256:(mc + 1) * 256],
                        rqk[:, 2 * h + mc: 1 + 2 * h + mc])
                negmax = wrk.tile([128, 2], f32, tag="negmax", bufs=2,
                                  name=f"nm{h}")
                rowsum = wrk.tile([128, 2], f32, tag="rowsum", bufs=2,
                                  name=f"rs{h}")
                recip = wrk.tile([128, 2], f32, tag=f"recip{h}",
                                 name=f"rc{h}")
                esb = wrk.tile([128, 512], f32, tag="esb", bufs=2,
                               name=f"esb{h}")
                for mc in range(2):
                    nc.vector.reduce_max(negmax[:, mc:mc + 1],
                                         sft[:, mc * 256:(mc + 1) * 256],
                                         axis=AX, negate=True)
                    nc.scalar.activation(esb[:, mc * 256:(mc + 1) * 256],
                                         sft[:, mc * 256:(mc + 1) * 256],
                                         Exp, bias=negmax[:, mc:mc + 1],
                                         accum_out=rowsum[:, mc:mc + 1])
                nc.vector.reciprocal(recip[:], rowsum[:])
                # fold the outT fp8 scale (x16) into the softmax denominator
                nc.vector.tensor_scalar_mul(recip[:], recip[:], 16.0)
                atp = ps.tile([128, 512], f32, tag="q1", name=f"atp{h}")
                for md in range(2):
                    for mc in range(2):
                        nc.tensor.transpose(
                            atp[:, md * 256 + mc * 128: md * 256 + (mc + 1) * 128],
                            esb[:, mc * 256 + md * 128: mc * 256 + (md + 1) * 128],
                            ident[:])
                atn = wrk.tile([128, 2, 256], f8, tag=f"atn{h}",
                               name=f"atn{h}")
                for i in range(2):
                    nc.vector.tensor_copy(atn[:, i, :],
                                          atp[:, i * 256:(i + 1) * 256])
                atns.append(atn)
                recips.append(recip)
            # pass 2: out^T[c,:] = sum_d attn^T[d,c] v[d,:] (one DoubleRow
            # pass); psum copy-scales split scalar/vector (gpsimd can't)
            cnt = 0
            for h in range(H):
                atn, recip = atns[h], recips[h]
                for mc in range(2):
                    for tc in range(4):
                        tg = ["k0", "k1", "gA", "gB"][cnt % 4]
                        pp = ps if cnt % 4 < 2 else ps2
                        op = pp.tile([128, 512], f32, tag=tg,
                                     name=f"op{h}_{mc}_{tc}")
                        cnt += 1
                        nc.tensor.matmul(
                            op[:], atn[:, :, mc * 128:(mc + 1) * 128],
                            v_sb[h][:, :, tc * 512:(tc + 1) * 512],
                            start=True, stop=True, perf_mode=DR)
                        dst = outT[h][:, mc, tc * 512:(tc + 1) * 512]
                        if tc % 2 == 0:
                            nc.scalar.activation(dst, op[:], Ident,
                                                 scale=recip[:, mc:mc + 1])
                        else:
                            nc.vector.tensor_scalar_mul(dst, op[:],
                                                        recip[:, mc:mc + 1])

            # ---------------- phase C: projection + bias + residual -------
            for j in range(S):
                if j % 2 == 0:
                    pq = [ps.tile([128, 512], f32, tag=PT[q],
                                  name=f"pq{j}_{q}")[:] for q in range(4)]
                else:
                    pA = ps2.tile([128, 1024], f32, tag="gA", name=f"pA{j}")
                    pB = ps2.tile([128, 1024], f32, tag="gB", name=f"pB{j}")
                    pq = [pA[:, 0:512], pA[:, 512:1024],
                          pB[:, 0:512], pB[:, 512:1024]]
                for kt2 in range(4):
                    fl, ll = (kt2 == 0), (kt2 == 3)
                    wps = wp[:, 2 * kt2:2 * kt2 + 2, j * 128:(j + 1) * 128]
                    for q in range(4):
                        nc.tensor.matmul(
                            pq[q], wps,
                            outT[q][:, :, kt2 * 512:(kt2 + 1) * 512],
                            start=fl, stop=ll, perf_mode=DR)
                ystage = wrk.tile([128, NL], bf16, tag="ystage", bufs=2,
                                  name=f"ystage{j}")
                for q in range(4):
                    # y = psum/(WS*16) + (x residual + bias)  [bias folded
                    # into xr on the host]
                    nc.vector.scalar_tensor_tensor(
                        ystage[:, q * 512:(q + 1) * 512], pq[q],
                        1.0 / (WS * 16.0), xr[:, j, q * 512:(q + 1) * 512],
                        op0=MULT, op1=ADD)
                nc.sync.dma_start(yT_d[j * 128:(j + 1) * 128, :], ystage[:])

    nc.compile()
    return nc


def _get_nc():
    if "nc" not in _CACHE:
        _CACHE["nc"] = _build()
    return _CACHE["nc"]


def _out_rows(half):
    # torch transpose+reshape scramble: this core's y rows
    return np.concatenate(
        [h * 1024 + half * 512 + np.arange(512) for h in range(H)])


def _make_in_maps(x, Wqkv, Wproj, bproj, temperature):
    import ml_dtypes
    f8 = ml_dtypes.float8_e4m3
    bf = ml_dtypes.bfloat16

    x = np.ascontiguousarray(np.asarray(x, dtype=np.float32))
    Wqkv = np.asarray(Wqkv, dtype=np.float32)
    Wproj = np.asarray(Wproj, dtype=np.float32)
    bproj = np.asarray(bproj, dtype=np.float32).reshape(C)
    temp = np.asarray(temperature, dtype=np.float32).reshape(H)

    WqkvT = Wqkv.T                                # [C, 3C]
    wqk8 = (WqkvT[:, :2 * C] * WS).reshape(S, 128, 2 * C) \
        .transpose(1, 0, 2).astype(f8)
    wv8 = (WqkvT[:, 2 * C:] * WS).reshape(S, 128, C) \
        .transpose(1, 0, 2).astype(f8)
    wp8 = (Wproj.T * WS).reshape(S, 128, C).transpose(1, 0, 2).astype(f8)
    tmpv2d = np.ascontiguousarray(np.repeat(temp, HD).reshape(S, 128).T)

    # store position p holds original local token t = 4*(p%512) + p//512 so
    # the proj-phase moving operands are contiguous
    tmap = 4 * (np.arange(NL) % 512) + np.arange(NL) // 512

    in_maps = []
    for core in range(NCORES):
        b, half = core // 2, core % 2
        xl = x[b, half * NL:(half + 1) * NL, :]   # [NL, C]
        x8 = xl[tmap, :].T.reshape(S, 128, NL).transpose(1, 0, 2).astype(f8)
        rows = _out_rows(half)
        # residual with the proj bias folded in (per y channel = xr row)
        xrb = (x[b, rows, :] + bproj[None, :]).T.astype(bf)   # [C, NL]
        in_maps.append(dict(x8=x8, xr=np.ascontiguousarray(xrb),
                            wqk8=wqk8, wv8=wv8, wp8=wp8, tmpv=tmpv2d))
    return in_maps


def _run(in_maps, trace=False, **kw):
    from concourse.bass_utils import run_bass_kernel_spmd

    nc = _get_nc()
    return run_bass_kernel_spmd(nc, in_maps, core_ids=list(range(NCORES)),
                                trace=trace, **kw)


def kernel(x, Wqkv, Wproj, bproj, temperature):
    res = _run(_make_in_maps(x, Wqkv, Wproj, bproj, temperature))
    y = np.empty((B, N, C), dtype=np.float32)
    for core in range(NCORES):
        b, half = core // 2, core % 2
        y[b, _out_rows(half), :] = res.results[core]["yT"].T.astype(np.float32)
    return y


# revision 13
# speedup vs baseline: 1.1251x; 1.1251x over previous
"""Efficient Channel Attention kernel for 8 Trainium2 NeuronCores.

Problem (B=4, N=4096, C=1024, H=4, HD=256):
    qkv = x @ Wqkv.T                 -> q,k,v per head, [HD, N] layout
    q,k l2-normalized over N; scores = (q*temp) @ k.T   [HD, HD] per (b,h)
    attn = softmax(scores, -1); out = attn @ v; y = out @ Wproj.T + bproj + x

Sharding: core = (batch b, token-half); only cross-core data is the
token-contracted Grams k^T q + q/k sumsq, AllReduce'd (f32, ~1MB) within
the core pair sharing a batch.

All heavy GEMMs run as fp8e4m3 DoubleRow matmuls (2 slab-contraction per
pass = 157 TF/s, 2x the f32r rate). Host pre-quantizes x to fp8 and the
weights to fp8*64 (keeps W entries out of fp8 subnormals); scales unwind in
the psum->SBUF copies. Local tokens are host-permuted t -> (t%4)*512 + t//4
so the torch transpose+reshape channel scramble makes the proj-phase moving
operands contiguous (the f32r baseline paid 1.65x for stride-4 reads).
Residual + y I/O in bf16.

PSUM tags: q0,q1,k0,k1 (4x1 bank) + gA,gB (2x2 banks) cover all phases.
"""

import numpy as np

B, N, C, H = 4, 4096, 1024, 4
HD = C // H          # 256
NCORES = 8
NL = N // 2          # 2048 tokens per core
S = C // 128         # 8 channel slabs
NT = NL // 128       # 16 token tiles
EPS = 1e-12
WS = 64.0            # host weight scale (fp8 subnormal dodge)
CCN = 128 * 2048 + 2 * C

_CACHE = {}


def _build():
    import concourse.mybir as mybir
    import concourse.tile as tile
    from concourse import bacc
    from concourse.masks import make_identity

    f32 = mybir.dt.float32
    bf16 = mybir.dt.bfloat16
    f8 = mybir.dt.float8e4
    AX = mybir.AxisListType.X
    ADD = mybir.AluOpType.add
    MULT = mybir.AluOpType.mult
    DR = mybir.MatmulPerfMode.DoubleRow
    Exp = mybir.ActivationFunctionType.Exp
    Ident = mybir.ActivationFunctionType.Identity
    Sqrt = mybir.ActivationFunctionType.Sqrt

    nc = bacc.Bacc("TRN2", target_bir_lowering=False, debug=False,
                   num_devices=NCORES)

    x8_d = nc.dram_tensor("x8", [128, S, NL], f8, kind="ExternalInput").ap()
    wqk_d = nc.dram_tensor("wqk8", [128, S, 2 * C], f8, kind="ExternalInput").ap()
    wv_d = nc.dram_tensor("wv8", [128, S, C], f8, kind="ExternalInput").ap()
    wp_d = nc.dram_tensor("wp8", [128, S, C], f8, kind="ExternalInput").ap()
    xr_d = nc.dram_tensor("xr", [C, NL], bf16, kind="ExternalInput").ap()
    tmpv_d = nc.dram_tensor("tmpv", [128, S], f32, kind="ExternalInput").ap()
    yT_d = nc.dram_tensor("yT", [C, NL], bf16, kind="ExternalOutput").ap()

    with tile.TileContext(nc) as tc:
        with (
            tc.tile_pool(name="const", bufs=1) as constp,
            tc.tile_pool(name="big", bufs=1) as bigp,
            tc.tile_pool(name="wrk", bufs=1) as wrk,
            tc.tile_pool(name="ps", bufs=1, space="PSUM") as ps,
            tc.tile_pool(name="ps2", bufs=1, space="PSUM") as ps2,
            tc.tile_pool(name="dram", bufs=1, space="DRAM") as dramp,
        ):
            PT = ["q0", "q1", "k0", "k1"]

            # ---------------- constants ----------------
            ident = constp.tile([128, 128], f32, name="ident")
            make_identity(nc, ident[:])
            tmpv_sb = constp.tile([128, S], f32, name="tmpv_sb")
            nc.sync.dma_start(tmpv_sb[:], tmpv_d[:])
            ones_sb = constp.tile([128, 1], bf16, name="ones_sb")
            nc.vector.memset(ones_sb[:], 1.0)

            # resident inputs; first slabs of x/wqk land first so A1 starts
            # x8 on the sync DGE queue, wqk on the Activation DGE queue —
            # the two 2MB loads stream in parallel so A1 starts early
            x8 = bigp.tile([128, S, NL], f8, name="x8")
            wqk = bigp.tile([128, S, 2 * C], f8, name="wqk")
            for s in range(S):
                nc.sync.dma_start(x8[:, s, :], x8_d[:, s, :])
                nc.scalar.dma_start(wqk[:, s, :], wqk_d[:, s, :])
            wv = bigp.tile([128, S, C], f8, name="wv")
            wp = bigp.tile([128, S, C], f8, name="wp")
            xr = bigp.tile([128, S, NL], bf16, name="xr")
            for s in range(S):
                nc.sync.dma_start(wv[:, s, :], wv_d[:, s, :])
            for s in range(S):
                nc.scalar.dma_start(wp[:, s, :], wp_d[:, s, :])

            # Gram accumulators: stA = heads 0,1 / stB = heads 2,3
            stA = ps2.tile([128, 1024], f32, tag="gA", name="stA")
            stB = ps2.tile([128, 1024], f32, tag="gB", name="stB")

            def st_slice(h, m):
                t = stA if h < 2 else stB
                off = (h % 2) * 512 + m * 256
                return t[:, off:off + 256]

            accq = wrk.tile([128, C], bf16, tag="accq", name="accq")
            acck = wrk.tile([128, C], bf16, tag="acck", name="acck")

            def _emit_gram(tp, qcol, kcol):
                for h in range(H):
                    for m in range(2):
                        nc.tensor.matmul(
                            st_slice(h, m),
                            kcol[:, :, h * 256 + m * 128: h * 256 + (m + 1) * 128],
                            qcol[:, :, h * 256:(h + 1) * 256],
                            start=(tp == 0), stop=(tp == NT // 2 - 1),
                            perf_mode=DR, skip_group_check=True)

            # ---------------- phase A1: q,k + Grams + sumsq ----------------
            _gram_pend = None
            for tp in range(NT // 2):
                qcol = wrk.tile([128, 2, C], f8, tag="qcol", bufs=2,
                                name=f"qcol{tp}")
                kcol = wrk.tile([128, 2, C], f8, tag="kcol", bufs=2,
                                name=f"kcol{tp}")
                for i2 in range(2):
                    tidx = tp * 2 + i2
                    t0 = tidx * 128
                    qp0 = ps.tile([128, 512], f32, tag="q0", name="qp0")
                    qp1 = ps.tile([128, 512], f32, tag="q1", name="qp1")
                    kp0 = ps.tile([128, 512], f32, tag="k0", name="kp0")
                    kp1 = ps.tile([128, 512], f32, tag="k1", name="kp1")
                    for kt2 in range(4):
                        lhs = x8[:, 2 * kt2:2 * kt2 + 2, t0:t0 + 128]
                        w2 = wqk[:, 2 * kt2:2 * kt2 + 2, :]
                        fl, ll = (kt2 == 0), (kt2 == 3)
                        nc.tensor.matmul(qp0[:], lhs, w2[:, :, 0:512],
                                         start=fl, stop=ll, perf_mode=DR)
                        nc.tensor.matmul(qp1[:], lhs, w2[:, :, 512:1024],
                                         start=fl, stop=ll, perf_mode=DR)
                        nc.tensor.matmul(kp0[:], lhs, w2[:, :, 1024:1536],
                                         start=fl, stop=ll, perf_mode=DR)
                        nc.tensor.matmul(kp1[:], lhs, w2[:, :, 1536:2048],
                                         start=fl, stop=ll, perf_mode=DR)
                    # fp8 copies for the Gram (vector); squares of the fp8
                    # values for the norms (scalar q / gpsimd k, from SBUF —
                    # gpsimd cannot touch PSUM), accumulate sumsq
                    nc.vector.tensor_scalar_mul(qcol[:, i2, 0:512], qp0[:],
                                                1.0 / WS)
                    nc.vector.tensor_scalar_mul(qcol[:, i2, 512:1024], qp1[:],
                                                1.0 / WS)
                    nc.vector.tensor_scalar_mul(kcol[:, i2, 0:512], kp0[:],
                                                1.0 / WS)
                    nc.vector.tensor_scalar_mul(kcol[:, i2, 512:1024], kp1[:],
                                                1.0 / WS)
                    sq = wrk.tile([128, C], bf16, tag="sq", bufs=2,
                                  name=f"sq{tidx}")
                    sk = wrk.tile([128, C], bf16, tag="sk", bufs=2,
                                  name=f"sk{tidx}")
                    nc.scalar.square(sq[:], qcol[:, i2, :])
                    nc.scalar.square(sk[:], kcol[:, i2, :])
                    if tidx == 0:
                        nc.gpsimd.tensor_copy(accq[:], sq[:])
                        nc.gpsimd.tensor_copy(acck[:], sk[:])
                    else:
                        nc.gpsimd.tensor_add(accq[:], accq[:], sq[:])
                        nc.gpsimd.tensor_add(acck[:], acck[:], sk[:])
                # Gram for the PREVIOUS pair: its fp8 copies finished long
                # ago, so the PE never stalls on fresh vector output
                if tp > 0:
                    _emit_gram(tp - 1, *_gram_pend)
                _gram_pend = (qcol, kcol)
            _emit_gram(NT // 2 - 1, *_gram_pend)

            # sumsq rows: [1, 512] ones-matmuls into the freed qk psum slots
            ss_ps = []
            for i, (src, lo) in enumerate([(accq, 0), (accq, 512),
                                           (acck, 0), (acck, 512)]):
                sp = ps.tile([1, 512], f32, tag=PT[i], name=f"ss{i}")
                nc.tensor.matmul(sp[:], ones_sb[:], src[:, lo:lo + 512],
                                 start=True, stop=True)
                ss_ps.append(sp)

            # SBUF bounces for the collective (DMA cannot read PSUM)
            # NOTE: bf16 collective hangs the runtime in-context (probed OK
            # bare); f32 collective is reliable — keep f32.
            ccdt = f32
            stA_sb = wrk.tile([128, 1024], ccdt, tag="ccA", name="stA_sb")
            stB_sb = wrk.tile([128, 1024], ccdt, tag="ccB", name="stB_sb")
            nc.vector.tensor_copy(stA_sb[:], stA[:])
            nc.vector.tensor_copy(stB_sb[:], stB[:])
            ss_sb = []
            for i in range(4):
                sb = wrk.tile([1, 512], ccdt, tag=f"ssb{i}", name=f"ssb{i}")
                nc.vector.tensor_copy(sb[:], ss_ps[i][:])
                ss_sb.append(sb)

            # ---------------- AllReduce over batch-pairs (bf16) -----------
            cc_in = dramp.tile([CCN], ccdt, name="cc_in")
            cc_out = dramp.tile([CCN], ccdt, name="cc_out")
            nc.sync.dma_start(
                cc_in[0:131072].rearrange("(p f) -> p f", p=128), stA_sb[:])
            nc.sync.dma_start(
                cc_in[131072:262144].rearrange("(p f) -> p f", p=128), stB_sb[:])
            for i in range(4):
                nc.sync.dma_start(
                    cc_in[262144 + i * 512: 262144 + (i + 1) * 512]
                    .rearrange("(a f) -> a f", a=1), ss_sb[i][:])
            nc.gpsimd.collective_compute(
                "AllReduce", ADD,
                replica_groups=[[0, 1], [2, 3], [4, 5], [6, 7]],
                ins=[cc_in.opt()], outs=[cc_out.opt()])
            # xr issued only now (Activation queue): its 4MB streams during
            # phase B instead of colliding with the AllReduce ring
            for s in range(S):
                nc.scalar.dma_start(xr[:, s, :], xr_d[s * 128:(s + 1) * 128, :])
            strA = wrk.tile([128, 1024], ccdt, tag="ccA", name="strA")
            strB = wrk.tile([128, 1024], ccdt, tag="ccB", name="strB")
            nc.sync.dma_start(
                strA[:], cc_out[0:131072].rearrange("(p f) -> p f", p=128))
            nc.sync.dma_start(
                strB[:], cc_out[131072:262144].rearrange("(p f) -> p f", p=128))
            ssred = constp.tile([128, 16], ccdt, name="ssred")
            nc.sync.dma_start(
                ssred[:],
                cc_out[262144:262144 + 2048].rearrange("(j p) -> p j", p=128))

            def str_slice(h, m):
                t = strA if h < 2 else strB
                off = (h % 2) * 512 + m * 256
                return t[:, off:off + 256]

            # ---------------- phase A2: v (overlaps the collective) -------
            v_sb = [bigp.tile([128, 2, NL], f8, name=f"v{h}") for h in range(H)]
            for vb in range(8):
                h, iv = vb // 2, vb % 2
                if vb % 2 == 0:
                    vps = [ps.tile([128, 512], f32, tag=PT[tc],
                                   name=f"vp{vb}_{tc}")[:] for tc in range(4)]
                else:
                    vA = ps2.tile([128, 1024], f32, tag="gA", name=f"vA{vb}")
                    vB = ps2.tile([128, 1024], f32, tag="gB", name=f"vB{vb}")
                    vps = [vA[:, 0:512], vA[:, 512:1024],
                           vB[:, 0:512], vB[:, 512:1024]]
                for kt2 in range(4):
                    fl, ll = (kt2 == 0), (kt2 == 3)
                    wvs = wv[:, 2 * kt2:2 * kt2 + 2, vb * 128:(vb + 1) * 128]
                    for tc in range(4):
                        nc.tensor.matmul(
                            vps[tc], wvs,
                            x8[:, 2 * kt2:2 * kt2 + 2, tc * 512:(tc + 1) * 512],
                            start=fl, stop=ll, perf_mode=DR)
                for tc in range(4):
                    nc.vector.tensor_scalar_mul(
                        v_sb[h][:, iv, tc * 512:(tc + 1) * 512], vps[tc],
                        1.0 / WS)

            # ---------------- phase B: normalize + softmax + attn@v -------
            # rq = temp/max(||q||,eps), rk = 1/max(||k||,eps) per channel:
            # rqk [128, 16]: cols 0-7 = rq (chan j*128+p), 8-15 = rk.
            rqk = constp.tile([128, 16], f32, name="rqk")
            nc.scalar.activation(rqk[:], ssred[:], Sqrt)
            nc.vector.tensor_scalar_max(rqk[:], rqk[:], EPS)
            nc.vector.reciprocal(rqk[:], rqk[:])
            nc.vector.tensor_mul(rqk[:, 0:8], rqk[:, 0:8], tmpv_sb[:])
            # preload the Exp activation table while A2 still owns the PE
            scrap = constp.tile([1, 1], f32, name="scrap")
            nc.scalar.activation(scrap[:], rqk[0:1, 0:1], Exp)

            outT = [bigp.tile([128, 2, NL], f8, name=f"ot{h}") for h in range(H)]

            cnt = 0

            def _emit_attnv(h, atn, recip):
                # out^T[c,:] = sum_d attn^T[d,c] v[d,:] (one DoubleRow pass);
                # psum copy-scales split scalar/vector (gpsimd can't)
                nonlocal cnt
                for mc in range(2):
                    for tc in range(4):
                        tg = ["k0", "k1", "gA", "gB"][cnt % 4]
                        pp = ps if cnt % 4 < 2 else ps2
                        op = pp.tile([128, 512], f32, tag=tg,
                                     name=f"op{h}_{mc}_{tc}")
                        cnt += 1
                        nc.tensor.matmul(
                            op[:], atn[:, :, mc * 128:(mc + 1) * 128],
                            v_sb[h][:, :, tc * 512:(tc + 1) * 512],
                            start=True, stop=True, perf_mode=DR)
                        dst = outT[h][:, mc, tc * 512:(tc + 1) * 512]
                        if tc % 2 == 0:
                            nc.scalar.activation(dst, op[:], Ident,
                                                 scale=recip[:, mc:mc + 1])
                        else:
                            nc.vector.tensor_scalar_mul(dst, op[:],
                                                        recip[:, mc:mc + 1])

            # per-head softmax -> fp8 attn^T; heads processed in 2-head
            # blocks with attn@v trailing, so the scalar engine's Exp/Ident
            # activation-table switches stay coarse while the attn@v matmuls
            # fill the next block's softmax-chain PE gaps
            _pass1_out = {}
            for h in range(H):
                # Gram^T rows d scaled by rk[d]
                sth = wrk.tile([128, 512], f32, tag="sth", bufs=2,
                               name=f"sth{h}")
                for m in range(2):
                    nc.gpsimd.tensor_scalar_mul(
                        sth[:, m * 256:(m + 1) * 256], str_slice(h, m),
                        rqk[:, 8 + 2 * h + m: 9 + 2 * h + m])
                spm = ps.tile([128, 512], f32, tag="q0", name=f"spm{h}")
                for mc in range(2):
                    for md in range(2):
                        nc.tensor.transpose(
                            spm[:, mc * 256 + md * 128: mc * 256 + (md + 1) * 128],
                            sth[:, md * 256 + mc * 128: md * 256 + (mc + 1) * 128],
                            ident[:])
                sft = wrk.tile([128, 512], f32, tag="sft", bufs=2,
                               name=f"sft{h}")
                for mc in range(2):
                    nc.vector.tensor_scalar_mul(
                        sft[:, mc * 256:(mc + 1) * 256],
                        spm[:, mc * 256:(mc + 1) * 256],
                        rqk[:, 2 * h + mc: 1 + 2 * h + mc])
                negmax = wrk.tile([128, 2], f32, tag="negmax", bufs=2,
                                  name=f"nm{h}")
                rowsum = wrk.tile([128, 2], f32, tag="rowsum", bufs=2,
                                  name=f"rs{h}")
                recip = wrk.tile([128, 2], f32, tag=f"recip{h}",
                                 name=f"rc{h}")
                esb = wrk.tile([128, 512], f32, tag="esb", bufs=2,
                               name=f"esb{h}")
                for mc in range(2):
                    nc.vector.reduce_max(negmax[:, mc:mc + 1],
                                         sft[:, mc * 256:(mc + 1) * 256],
                                         axis=AX, negate=True)
                    nc.scalar.activation(esb[:, mc * 256:(mc + 1) * 256],
                                         sft[:, mc * 256:(mc + 1) * 256],
                                         Exp, bias=negmax[:, mc:mc + 1],
                                         accum_out=rowsum[:, mc:mc + 1])
                nc.vector.reciprocal(recip[:], rowsum[:])
                # fold the outT fp8 scale (x16) into the softmax denominator
                nc.vector.tensor_scalar_mul(recip[:], recip[:], 16.0)
                atp = ps.tile([128, 512], f32, tag="q1", name=f"atp{h}")
                for md in range(2):
                    for mc in range(2):
                        nc.tensor.transpose(
                            atp[:, md * 256 + mc * 128: md * 256 + (mc + 1) * 128],
                            esb[:, mc * 256 + md * 128: mc * 256 + (md + 1) * 128],
                            ident[:])
                atn = wrk.tile([128, 2, 256], f8, tag=f"atn{h}",
                               name=f"atn{h}")
                for i in range(2):
                    nc.vector.tensor_copy(atn[:, i, :],
                                          atp[:, i * 256:(i + 1) * 256])
                _pass1_out[h] = (atn, recip)
                if h == 1:
                    pass  # attnv for block 0 emitted after pass1 h=2 below
                if h == 2:
                    _emit_attnv(0, *_pass1_out[0])
                    _emit_attnv(1, *_pass1_out[1])
            _emit_attnv(2, *_pass1_out[2])
            _emit_attnv(3, *_pass1_out[3])

            # ---------------- phase C: projection + bias + residual -------
            for j in range(S):
                if j % 2 == 0:
                    pq = [ps.tile([128, 512], f32, tag=PT[q],
                                  name=f"pq{j}_{q}")[:] for q in range(4)]
                else:
                    pA = ps2.tile([128, 1024], f32, tag="gA", name=f"pA{j}")
                    pB = ps2.tile([128, 1024], f32, tag="gB", name=f"pB{j}")
                    pq = [pA[:, 0:512], pA[:, 512:1024],
                          pB[:, 0:512], pB[:, 512:1024]]
                for kt2 in range(4):
                    fl, ll = (kt2 == 0), (kt2 == 3)
                    wps = wp[:, 2 * kt2:2 * kt2 + 2, j * 128:(j + 1) * 128]
                    for q in range(4):
                        nc.tensor.matmul(
                            pq[q], wps,
                            outT[q][:, :, kt2 * 512:(kt2 + 1) * 512],
                            start=fl, stop=ll, perf_mode=DR)
                ystage = wrk.tile([128, NL], bf16, tag="ystage", bufs=2,
                                  name=f"ystage{j}")
                for q in range(4):
                    # y = psum/(WS*16) + (x residual + bias)  [bias folded
                    # into xr on the host]
                    nc.vector.scalar_tensor_tensor(
                        ystage[:, q * 512:(q + 1) * 512], pq[q],
                        1.0 / (WS * 16.0), xr[:, j, q * 512:(q + 1) * 512],
                        op0=MULT, op1=ADD)
                eng = nc.sync if j % 2 == 0 else nc.scalar
                eng.dma_start(yT_d[j * 128:(j + 1) * 128, :], ystage[:])

    nc.compile()
    return nc


def _get_nc():
    if "nc" not in _CACHE:
        _CACHE["nc"] = _build()
    return _CACHE["nc"]


def _out_rows(half):
    # torch transpose+reshape scramble: this core's y rows
    return np.concatenate(
        [h * 1024 + half * 512 + np.arange(512) for h in range(H)])


def _make_in_maps(x, Wqkv, Wproj, bproj, temperature):
    import ml_dtypes
    f8 = ml_dtypes.float8_e4m3
    bf = ml_dtypes.bfloat16

    x = np.ascontiguousarray(np.asarray(x, dtype=np.float32))
    Wqkv = np.asarray(Wqkv, dtype=np.float32)
    Wproj = np.asarray(Wproj, dtype=np.float32)
    bproj = np.asarray(bproj, dtype=np.float32).reshape(C)
    temp = np.asarray(temperature, dtype=np.float32).reshape(H)

    WqkvT = Wqkv.T                                # [C, 3C]
    wqk8 = (WqkvT[:, :2 * C] * WS).reshape(S, 128, 2 * C) \
        .transpose(1, 0, 2).astype(f8)
    wv8 = (WqkvT[:, 2 * C:] * WS).reshape(S, 128, C) \
        .transpose(1, 0, 2).astype(f8)
    wp8 = (Wproj.T * WS).reshape(S, 128, C).transpose(1, 0, 2).astype(f8)
    tmpv2d = np.ascontiguousarray(np.repeat(temp, HD).reshape(S, 128).T)

    # store position p holds original local token t = 4*(p%512) + p//512 so
    # the proj-phase moving operands are contiguous
    tmap = 4 * (np.arange(NL) % 512) + np.arange(NL) // 512

    in_maps = []
    for core in range(NCORES):
        b, half = core // 2, core % 2
        xl = x[b, half * NL:(half + 1) * NL, :]   # [NL, C]
        x8 = xl[tmap, :].T.reshape(S, 128, NL).transpose(1, 0, 2).astype(f8)
        rows = _out_rows(half)
        # residual with the proj bias folded in (per y channel = xr row)
        xrb = (x[b, rows, :] + bproj[None, :]).T.astype(bf)   # [C, NL]
        in_maps.append(dict(x8=x8, xr=np.ascontiguousarray(xrb),
                            wqk8=wqk8, wv8=wv8, wp8=wp8, tmpv=tmpv2d))
    return in_maps


def _run(in_maps, trace=False, **kw):
    from concourse.bass_utils import run_bass_kernel_spmd

    nc = _get_nc()
    return run_bass_kernel_spmd(nc, in_maps, core_ids=list(range(NCORES)),
                                trace=trace, **kw)


def kernel(x, Wqkv, Wproj, bproj, temperature):
    res = _run(_make_in_maps(x, Wqkv, Wproj, bproj, temperature))
    y = np.empty((B, N, C), dtype=np.float32)
    for core in range(NCORES):
        b, half = core // 2, core % 2
        y[b, _out_rows(half), :] = res.results[core]["yT"].T.astype(np.float32)
    return y


# revision 16
# speedup vs baseline: 1.2060x; 1.0719x over previous
"""Efficient Channel Attention kernel for 8 Trainium2 NeuronCores.

Problem (B=4, N=4096, C=1024, H=4, HD=256):
    qkv = x @ Wqkv.T                 -> q,k,v per head, [HD, N] layout
    q,k l2-normalized over N; scores = (q*temp) @ k.T   [HD, HD] per (b,h)
    attn = softmax(scores, -1); out = attn @ v; y = out @ Wproj.T + bproj + x

Sharding: core = (batch b, token-half); the only cross-core data is the
token-contracted Grams k^T q + q/k sumsq, AllReduce'd within the core pair
sharing a batch.

All heavy GEMMs run as fp8e4m3 DoubleRow matmuls (2-slab contraction per
pass = 157 TF/s, 2x the f32r rate; measured 216ns per 512-moving matmul).
Host pre-quantizes x to fp8 and weights to fp8*64 (dodges fp8 subnormals);
scales unwind in the psum->SBUF copies. Local tokens are host-permuted
t -> (t%4)*512 + t//4 so the torch transpose+reshape channel scramble makes
the proj-phase moving operands contiguous. Residual (+folded proj bias) and
y I/O in bf16.

Engine budget per A1 token-tile (PE 3.46us): vector 2 psum->fp8 copies
(~2.5us), scalar 2 squares (~2.3us), sumsq accumulation via DMA accum_op on
the otherwise-idle sync queue, gpsimd free. Softmax needs no max-subtract:
|score| <= temp (Cauchy-Schwarz on unit vectors).

PSUM: four 2-bank slots (bQ,bK / gA,gB) cover all phases.
"""

import numpy as np

B, N, C, H = 4, 4096, 1024, 4
HD = C // H          # 256
NCORES = 8
NL = N // 2          # 2048 tokens per core
S = C // 128         # 8 channel slabs
NT = NL // 128       # 16 token tiles
EPS = 1e-12
WS = 64.0            # host weight scale (fp8 subnormal dodge)
CCN = 128 * 2048 + 2 * C
CC_BF16 = True       # AllReduce payload dtype (False -> f32 fallback)

_CACHE = {}


def _build():
    import concourse.mybir as mybir
    import concourse.tile as tile
    from concourse import bacc
    from concourse.masks import make_identity

    f32 = mybir.dt.float32
    bf16 = mybir.dt.bfloat16
    f8 = mybir.dt.float8e4
    ccdt = bf16 if CC_BF16 else f32
    AX = mybir.AxisListType.X
    ADD = mybir.AluOpType.add
    MULT = mybir.AluOpType.mult
    DR = mybir.MatmulPerfMode.DoubleRow
    Exp = mybir.ActivationFunctionType.Exp
    Ident = mybir.ActivationFunctionType.Identity
    Sqrt = mybir.ActivationFunctionType.Sqrt

    nc = bacc.Bacc("TRN2", target_bir_lowering=False, debug=False,
                   num_devices=NCORES)

    x8_d = nc.dram_tensor("x8", [128, S, NL], f8, kind="ExternalInput").ap()
    wqk_d = nc.dram_tensor("wqk8", [128, S, 2 * C], f8, kind="ExternalInput").ap()
    wv_d = nc.dram_tensor("wv8", [128, S, C], f8, kind="ExternalInput").ap()
    wp_d = nc.dram_tensor("wp8", [128, S, C], f8, kind="ExternalInput").ap()
    xr_d = nc.dram_tensor("xr", [C, NL], bf16, kind="ExternalInput").ap()
    tmpv_d = nc.dram_tensor("tmpv", [128, S], f32, kind="ExternalInput").ap()
    yT_d = nc.dram_tensor("yT", [C, NL], bf16, kind="ExternalOutput").ap()

    with tile.TileContext(nc) as tc:
        with (
            tc.tile_pool(name="const", bufs=1) as constp,
            tc.tile_pool(name="big", bufs=1) as bigp,
            tc.tile_pool(name="wrk", bufs=1) as wrk,
            tc.tile_pool(name="ps", bufs=1, space="PSUM") as ps,
            tc.tile_pool(name="ps2", bufs=1, space="PSUM") as ps2,
            tc.tile_pool(name="dram", bufs=1, space="DRAM") as dramp,
        ):
            # ---------------- constants ----------------
            ident = constp.tile([128, 128], f32, name="ident")
            make_identity(nc, ident[:])
            tmpv_sb = constp.tile([128, S], f32, name="tmpv_sb")
            nc.sync.dma_start(tmpv_sb[:], tmpv_d[:])
            ones_sb = constp.tile([128, 1], bf16, name="ones_sb")
            nc.vector.memset(ones_sb[:], 1.0)

            # x8 on the sync DGE queue, wqk on the Activation DGE queue --
            # the two 2MB loads stream in parallel so A1 starts early
            x8 = bigp.tile([128, S, NL], f8, name="x8")
            wqk = bigp.tile([128, S, 2 * C], f8, name="wqk")
            for s in range(S):
                nc.sync.dma_start(x8[:, s, :], x8_d[:, s, :])
                nc.scalar.dma_start(wqk[:, s, :], wqk_d[:, s, :])
            wv = bigp.tile([128, S, C], f8, name="wv")
            wp = bigp.tile([128, S, C], f8, name="wp")
            xr = bigp.tile([128, S, NL], bf16, name="xr")
            for s in range(S):
                nc.sync.dma_start(wv[:, s, :], wv_d[:, s, :])
            for s in range(S):
                nc.scalar.dma_start(wp[:, s, :], wp_d[:, s, :])

            # Gram accumulators: stA = heads 0,1 / stB = heads 2,3
            stA = ps2.tile([128, 1024], f32, tag="gA", name="stA")
            stB = ps2.tile([128, 1024], f32, tag="gB", name="stB")

            def st_slice(h, m):
                t = stA if h < 2 else stB
                off = (h % 2) * 512 + m * 256
                return t[:, off:off + 256]

            accq = wrk.tile([128, C], bf16, tag="accq", name="accq")
            acck = wrk.tile([128, C], bf16, tag="acck", name="acck")

            def _emit_gram(tp, qcol, kcol):
                for h in range(H):
                    for m in range(2):
                        nc.tensor.matmul(
                            st_slice(h, m),
                            kcol[:, :, h * 256 + m * 128: h * 256 + (m + 1) * 128],
                            qcol[:, :, h * 256:(h + 1) * 256],
                            start=(tp == 0), stop=(tp == NT // 2 - 1),
                            perf_mode=DR, skip_group_check=True)

            # ---------------- phase A1: q,k + Grams + sumsq ----------------
            _gram_pend = None
            for tp in range(NT // 2):
                qcol = wrk.tile([128, 2, C], f8, tag="qcol", bufs=2,
                                name=f"qcol{tp}")
                kcol = wrk.tile([128, 2, C], f8, tag="kcol", bufs=2,
                                name=f"kcol{tp}")
                for i2 in range(2):
                    tidx = tp * 2 + i2
                    t0 = tidx * 128
                    qp = ps.tile([128, 1024], f32, tag="bQ", name="qp")
                    kp = ps.tile([128, 1024], f32, tag="bK", name="kp")
                    for kt2 in range(4):
                        lhs = x8[:, 2 * kt2:2 * kt2 + 2, t0:t0 + 128]
                        w2 = wqk[:, 2 * kt2:2 * kt2 + 2, :]
                        fl, ll = (kt2 == 0), (kt2 == 3)
                        nc.tensor.matmul(qp[:, 0:512], lhs, w2[:, :, 0:512],
                                         start=fl, stop=ll, perf_mode=DR)
                        nc.tensor.matmul(qp[:, 512:1024], lhs,
                                         w2[:, :, 512:1024],
                                         start=fl, stop=ll, perf_mode=DR)
                        nc.tensor.matmul(kp[:, 0:512], lhs,
                                         w2[:, :, 1024:1536],
                                         start=fl, stop=ll, perf_mode=DR)
                        nc.tensor.matmul(kp[:, 512:1024], lhs,
                                         w2[:, :, 1536:2048],
                                         start=fl, stop=ll, perf_mode=DR)
                    # fp8 copies for the Gram (vector), squares of the raw
                    # psum (scalar; 64^2 scale folds into the norm later),
                    # sumsq accumulated by the DMA ALU on the idle sync queue
                    nc.vector.tensor_scalar_mul(qcol[:, i2, :], qp[:],
                                                1.0 / WS)
                    nc.vector.tensor_scalar_mul(kcol[:, i2, :], kp[:],
                                                1.0 / WS)
                    sq = wrk.tile([128, C], bf16, tag="sq", bufs=2,
                                  name=f"sq{tidx}")
                    sk = wrk.tile([128, C], bf16, tag="sk", bufs=2,
                                  name=f"sk{tidx}")
                    nc.scalar.square(sq[:], qp[:])
                    nc.scalar.square(sk[:], kp[:])
                    op_acc = mybir.AluOpType.bypass if tidx == 0 else ADD
                    nc.gpsimd.dma_start(accq[:], sq[:], accum_op=op_acc)
                    nc.gpsimd.dma_start(acck[:], sk[:], accum_op=op_acc)
                # Gram for the PREVIOUS pair: its fp8 copies finished long
                # ago, so the PE never stalls on fresh vector output
                if tp > 0:
                    _emit_gram(tp - 1, *_gram_pend)
                _gram_pend = (qcol, kcol)
            _emit_gram(NT // 2 - 1, *_gram_pend)

            # preload the Sqrt table while the PE streams A2
            scrap = constp.tile([1, 1], f32, name="scrap")
            nc.scalar.activation(scrap[:], tmpv_sb[0:1, 0:1], Sqrt)

            # SBUF bounces for the collective (DMA cannot read PSUM)
            stA_sb = wrk.tile([128, 1024], ccdt, tag="ccA", name="stA_sb")
            stB_sb = wrk.tile([128, 1024], ccdt, tag="ccB", name="stB_sb")
            nc.vector.tensor_copy(stA_sb[:], stA[:])
            nc.vector.tensor_copy(stB_sb[:], stB[:])
            cc_in = dramp.tile([CCN], ccdt, name="cc_in")
            cc_out = dramp.tile([CCN], ccdt, name="cc_out")
            nc.sync.dma_start(
                cc_in[0:131072].rearrange("(p f) -> p f", p=128), stA_sb[:])
            nc.sync.dma_start(
                cc_in[131072:262144].rearrange("(p f) -> p f", p=128), stB_sb[:])

            # ---------------- phase A2: v (overlaps the collective) -------
            v_sb = [bigp.tile([128, 2, NL], f8, name=f"v{h}") for h in range(H)]
            PT2 = [("bQ", ps), ("bK", ps), ("gA", ps2), ("gB", ps2)]

            def _emit_v(vb):
                h, iv = vb // 2, vb % 2
                tg1, pp1 = PT2[(2 * vb) % 4]
                tg2, pp2 = PT2[(2 * vb + 1) % 4]
                vpa = pp1.tile([128, 1024], f32, tag=tg1, name=f"vA{vb}")
                vpb = pp2.tile([128, 1024], f32, tag=tg2, name=f"vB{vb}")
                vps = [vpa[:, 0:512], vpa[:, 512:1024],
                       vpb[:, 0:512], vpb[:, 512:1024]]
                for kt2 in range(4):
                    fl, ll = (kt2 == 0), (kt2 == 3)
                    wvs = wv[:, 2 * kt2:2 * kt2 + 2, vb * 128:(vb + 1) * 128]
                    for tc in range(4):
                        nc.tensor.matmul(
                            vps[tc], wvs,
                            x8[:, 2 * kt2:2 * kt2 + 2, tc * 512:(tc + 1) * 512],
                            start=fl, stop=ll, perf_mode=DR)
                nc.vector.tensor_scalar_mul(v_sb[h][:, iv, 0:1024], vpa[:],
                                            1.0 / WS)
                nc.vector.tensor_scalar_mul(v_sb[h][:, iv, 1024:2048], vpb[:],
                                            1.0 / WS)

            _emit_v(0)
            _emit_v(1)

            # sumsq rows: ones-matmuls (bf16) into the gA/gB slots freed by
            # the stA/stB bounce copies; then the AllReduce fires
            ssq = ps2.tile([1, 1024], f32, tag="gA", name="ssq")
            ssk = ps2.tile([1, 1024], f32, tag="gB", name="ssk")
            for lo in (0, 512):
                nc.tensor.matmul(ssq[:, lo:lo + 512], ones_sb[:],
                                 accq[:, lo:lo + 512], start=True, stop=True)
                nc.tensor.matmul(ssk[:, lo:lo + 512], ones_sb[:],
                                 acck[:, lo:lo + 512], start=True, stop=True)
            ssq_sb = wrk.tile([1, 1024], ccdt, tag="ssqb", name="ssq_sb")
            ssk_sb = wrk.tile([1, 1024], ccdt, tag="sskb", name="ssk_sb")
            nc.vector.tensor_copy(ssq_sb[:], ssq[:])
            nc.vector.tensor_copy(ssk_sb[:], ssk[:])
            nc.sync.dma_start(
                cc_in[262144:263168].rearrange("(a f) -> a f", a=1), ssq_sb[:])
            nc.sync.dma_start(
                cc_in[263168:264192].rearrange("(a f) -> a f", a=1), ssk_sb[:])
            nc.gpsimd.collective_compute(
                "AllReduce", ADD,
                replica_groups=[[0, 1], [2, 3], [4, 5], [6, 7]],
                ins=[cc_in.opt()], outs=[cc_out.opt()])

            for vb in range(2, 8):
                _emit_v(vb)

            # xr issued only now (Activation queue): its 4MB streams during
            # phase B instead of colliding with the AllReduce ring
            for s in range(S):
                nc.scalar.dma_start(xr[:, s, :], xr_d[s * 128:(s + 1) * 128, :])

            # readback split across both DGE queues
            strA = wrk.tile([128, 1024], ccdt, tag="ccA", name="strA")
            strB = wrk.tile([128, 1024], ccdt, tag="ccB", name="strB")
            nc.sync.dma_start(
                strA[:], cc_out[0:131072].rearrange("(p f) -> p f", p=128))
            nc.scalar.dma_start(
                strB[:], cc_out[131072:262144].rearrange("(p f) -> p f", p=128))
            ssred = constp.tile([128, 16], ccdt, name="ssred")
            nc.sync.dma_start(
                ssred[:],
                cc_out[262144:264192].rearrange("(j p) -> p j", p=128))

            def str_slice(h, m):
                t = strA if h < 2 else strB
                off = (h % 2) * 512 + m * 256
                return t[:, off:off + 256]

            # ---------------- phase B: normalize + softmax + attn@v -------
            # rq = temp/max(||q||,eps), rk = 1/max(||k||,eps) per channel:
            # rqk [128, 16]: cols 0-7 = rq (chan j*128+p), 8-15 = rk.
            # ssred carries (64|q|)^2 sums -> scale 1/WS^2 inside the sqrt.
            rqk = constp.tile([128, 16], f32, name="rqk")
            nc.scalar.activation(rqk[:], ssred[:], Sqrt, scale=1.0 / (WS * WS))
            nc.vector.tensor_scalar_max(rqk[:], rqk[:], EPS)
            nc.vector.reciprocal(rqk[:], rqk[:])
            nc.vector.tensor_mul(rqk[:, 0:8], rqk[:, 0:8], tmpv_sb[:])
            # preload the Exp activation table before the first real softmax
            nc.scalar.activation(scrap[:], rqk[0:1, 0:1], Exp)

            outT = [bigp.tile([128, 2, NL], f8, name=f"ot{h}") for h in range(H)]
            cnt = 0

            def _emit_attnv(h, atn, recip):
                # out^T[c,:] = sum_d attn^T[d,c] v[d,:] (one DoubleRow pass);
                # psum copy-scales lean on scalar (5) vs vector (3)
                nonlocal cnt
                for mc in range(2):
                    for tc in range(4):
                        tg, pp = PT2[2 + (cnt // 2) % 2]
                        op = pp.tile([128, 1024], f32, tag=tg,
                                     name=f"op{h}_{mc}_{tc}")
                        opx = op[:, 0:512] if cnt % 2 == 0 else op[:, 512:1024]
                        cnt += 1
                        nc.tensor.matmul(
                            opx, atn[:, :, mc * 128:(mc + 1) * 128],
                            v_sb[h][:, :, tc * 512:(tc + 1) * 512],
                            start=True, stop=True, perf_mode=DR)
                        dst = outT[h][:, mc, tc * 512:(tc + 1) * 512]
                        if (mc * 4 + tc) % 8 < 5:
                            nc.scalar.activation(dst, opx, Ident,
                                                 scale=recip[:, mc:mc + 1])
                        else:
                            nc.vector.tensor_scalar_mul(dst, opx,
                                                        recip[:, mc:mc + 1])

            # per-head softmax -> fp8 attn^T; heads in 2-head blocks with
            # attn@v trailing so the scalar Exp/Ident table switches stay
            # coarse while attn@v matmuls fill the softmax-chain PE gaps
            _pass1_out = {}
            for h in range(H):
                # Gram^T rows d scaled by rk[d]
                sth = wrk.tile([128, 512], f32, tag="sth", bufs=2,
                               name=f"sth{h}")
                for m in range(2):
                    nc.vector.tensor_scalar_mul(
                        sth[:, m * 256:(m + 1) * 256], str_slice(h, m),
                        rqk[:, 8 + 2 * h + m: 9 + 2 * h + m])
                spm = ps.tile([128, 512], f32, tag="bQ", name=f"spm{h}")
                for mc in range(2):
                    for md in range(2):
                        nc.tensor.transpose(
                            spm[:, mc * 256 + md * 128: mc * 256 + (md + 1) * 128],
                            sth[:, md * 256 + mc * 128: md * 256 + (mc + 1) * 128],
                            ident[:])
                sft = wrk.tile([128, 512], f32, tag="sft", bufs=2,
                               name=f"sft{h}")
                for mc in range(2):
                    nc.vector.tensor_scalar_mul(
                        sft[:, mc * 256:(mc + 1) * 256],
                        spm[:, mc * 256:(mc + 1) * 256],
                        rqk[:, 2 * h + mc: 1 + 2 * h + mc])
                # no max-subtraction: |score| <= temp (unit vectors), exp is
                # safely bounded
                rowsum = wrk.tile([128, 2], f32, tag="rowsum", bufs=2,
                                  name=f"rs{h}")
                recip = wrk.tile([128, 2], f32, tag=f"recip{h}",
                                 name=f"rc{h}")
                esb = wrk.tile([128, 512], f32, tag="esb", bufs=2,
                               name=f"esb{h}")
                for mc in range(2):
                    nc.scalar.activation(esb[:, mc * 256:(mc + 1) * 256],
                                         sft[:, mc * 256:(mc + 1) * 256],
                                         Exp, accum_out=rowsum[:, mc:mc + 1])
                nc.vector.reciprocal(recip[:], rowsum[:])
                # fold the outT fp8 scale (x16) into the softmax denominator
                nc.vector.tensor_scalar_mul(recip[:], recip[:], 16.0)
                atp = ps.tile([128, 512], f32, tag="bK", name=f"atp{h}")
                for md in range(2):
                    for mc in range(2):
                        nc.tensor.transpose(
                            atp[:, md * 256 + mc * 128: md * 256 + (mc + 1) * 128],
                            esb[:, mc * 256 + md * 128: mc * 256 + (md + 1) * 128],
                            ident[:])
                # attn^T slabs are contiguous in atp: one flat copy
                atn = wrk.tile([128, 2, 256], f8, tag=f"atn{h}",
                               name=f"atn{h}")
                nc.vector.tensor_copy(atn[:, :, :], atp[:])
                _pass1_out[h] = (atn, recip)
                if h == 2:
                    _emit_attnv(0, *_pass1_out[0])
                    _emit_attnv(1, *_pass1_out[1])
            _emit_attnv(2, *_pass1_out[2])
            _emit_attnv(3, *_pass1_out[3])

            # ---------------- phase C: projection + residual --------------
            for j in range(S):
                tgA, ppA = PT2[(2 * j) % 4]
                tgB, ppB = PT2[(2 * j + 1) % 4]
                pA = ppA.tile([128, 1024], f32, tag=tgA, name=f"pA{j}")
                pB = ppB.tile([128, 1024], f32, tag=tgB, name=f"pB{j}")
                pq = [pA[:, 0:512], pA[:, 512:1024],
                      pB[:, 0:512], pB[:, 512:1024]]
                for kt2 in range(4):
                    fl, ll = (kt2 == 0), (kt2 == 3)
                    wps = wp[:, 2 * kt2:2 * kt2 + 2, j * 128:(j + 1) * 128]
                    for q in range(4):
                        nc.tensor.matmul(
                            pq[q], wps,
                            outT[q][:, :, kt2 * 512:(kt2 + 1) * 512],
                            start=fl, stop=ll, perf_mode=DR)
                ystage = wrk.tile([128, NL], bf16, tag="ystage", bufs=2,
                                  name=f"ystage{j}")
                for q in range(4):
                    # y = psum/(WS*16) + (x residual + bias)  [bias folded
                    # into xr on the host]
                    nc.vector.scalar_tensor_tensor(
                        ystage[:, q * 512:(q + 1) * 512], pq[q],
                        1.0 / (WS * 16.0), xr[:, j, q * 512:(q + 1) * 512],
                        op0=MULT, op1=ADD)
                eng = nc.sync if j % 2 == 0 else nc.scalar
                eng.dma_start(yT_d[j * 128:(j + 1) * 128, :], ystage[:])

    nc.compile()
    return nc


def _get_nc():
    if "nc" not in _CACHE:
        _CACHE["nc"] = _build()
    return _CACHE["nc"]


def _out_rows(half):
    # torch transpose+reshape scramble: this core's y rows
    return np.concatenate(
        [h * 1024 + half * 512 + np.arange(512) for h in range(H)])


def _make_in_maps(x, Wqkv, Wproj, bproj, temperature):
    import ml_dtypes
    f8 = ml_dtypes.float8_e4m3
    bf = ml_dtypes.bfloat16

    x = np.ascontiguousarray(np.asarray(x, dtype=np.float32))
    Wqkv = np.asarray(Wqkv, dtype=np.float32)
    Wproj = np.asarray(Wproj, dtype=np.float32)
    bproj = np.asarray(bproj, dtype=np.float32).reshape(C)
    temp = np.asarray(temperature, dtype=np.float32).reshape(H)

    WqkvT = Wqkv.T                                # [C, 3C]
    wqk8 = (WqkvT[:, :2 * C] * WS).reshape(S, 128, 2 * C) \
        .transpose(1, 0, 2).astype(f8)
    wv8 = (WqkvT[:, 2 * C:] * WS).reshape(S, 128, C) \
        .transpose(1, 0, 2).astype(f8)
    wp8 = (Wproj.T * WS).reshape(S, 128, C).transpose(1, 0, 2).astype(f8)
    tmpv2d = np.ascontiguousarray(np.repeat(temp, HD).reshape(S, 128).T)

    # store position p holds original local token t = 4*(p%512) + p//512 so
    # the proj-phase moving operands are contiguous
    tmap = 4 * (np.arange(NL) % 512) + np.arange(NL) // 512

    in_maps = []
    for core in range(NCORES):
        b, half = core // 2, core % 2
        xl = x[b, half * NL:(half + 1) * NL, :]   # [NL, C]
        x8 = xl[tmap, :].T.reshape(S, 128, NL).transpose(1, 0, 2).astype(f8)
        rows = _out_rows(half)
        # residual with the proj bias folded in (per y channel = xr row)
        xrb = (x[b, rows, :] + bproj[None, :]).T.astype(bf)   # [C, NL]
        in_maps.append(dict(x8=x8, xr=np.ascontiguousarray(xrb),
                            wqk8=wqk8, wv8=wv8, wp8=wp8, tmpv=tmpv2d))
    return in_maps


def _run(in_maps, trace=False, **kw):
    from concourse.bass_utils import run_bass_kernel_spmd

    nc = _get_nc()
    return run_bass_kernel_spmd(nc, in_maps, core_ids=list(range(NCORES)),
                                trace=trace, **kw)


def kernel(x, Wqkv, Wproj, bproj, temperature):
    res = _run(_make_in_maps(x, Wqkv, Wproj, bproj, temperature))
    y = np.empty((B, N, C), dtype=np.float32)
    for core in range(NCORES):
        b, half = core // 2, core % 2
        y[b, _out_rows(half), :] = res.results[core]["yT"].T.astype(np.float32)
    return y
